# revision 5
# baseline (speedup 1.0000x reference)
"""Trainium2 Bass kernel for nn_DIDAModuleD4 — v2 (fp8 DoubleRow taps).

Data-parallel over batch: 32 samples -> 8 cores x 4 samples, processed in
2 blocks of 2 samples; the 2x64 (sample, channel) pairs occupy the 128
SBUF partitions.

v2 moves ALL 43 depthwise taps onto the PE as fp8e4m3 DoubleRow
diag-matmul pairs (2 taps per instruction at 0.5 cyc/col), freeing
DVE/Pool almost entirely:
  conv:  fp16 matmuls, both samples packed into one [128,512] psum via
         output-partition offset; ACT evacuates to an fp8 padded-f tile
         with bias and accum_out (pre-quantization sums -> exact g).
  taps:  per 7-row slab, 504-wide contiguous windows (incl 8 junk cols
         per row); tap pairs share one DoubleRow matmul via a custom
         even-stride dim-1 AP; diag pairs built on Pool from ktile.
  out:   bf16 3-piece K=64 matmuls (as v1, og piece gone); psum
         evacuated to fp16 staging on ACT (s0) / DVE (s1); fp16 stores
         on the SP sequencer; host converts y to f32.
Datapath: x/conv fp16, fpad/diag fp8e4m3, o bf16, wout bf16, y fp16.
"""

import sys

if "/opt/trn_rl_repo" not in sys.path:
    sys.path.insert(0, "/opt/trn_rl_repo")

import os
import numpy as np
from contextlib import ExitStack

import bass_rust
from concourse import bass, mybir, tile, bacc
from concourse.bass_utils import run_bass_kernel_spmd

F32 = mybir.dt.float32
F16 = mybir.dt.float16
BF16 = mybir.dt.bfloat16
F8 = mybir.dt.float8e4
AF = mybir.ActivationFunctionType
ALU = mybir.AluOpType
DRMODE = mybir.MatmulPerfMode.DoubleRow

N_CORES = 8
SAMPLES_PER_CORE = 4
CM = 64
CIN = 256
COUT = 384
H = W = 64
PIX = H * W
PAD = 4
WP = W + 2 * PAD          # 72
IMG_BASE = 4              # guard elements before the padded image
FPW = IMG_BASE + WP * WP + 12   # 5200 per-partition fp8 elements
SLAB = 1024               # conv slab (pixels)
CHUNK = 512               # conv / output matmul N
NCHUNK = PIX // CHUNK     # 8
# tap slabs: 9 x 7 rows + 1 x 1 row
TSLABS = [(r, 7) for r in range(0, 63, 7)] + [(63, 1)]

TAPS = (
    [(0, dy, dx, 1) for dy in range(-2, 3) for dx in range(-2, 3)]
    + [(1, dy, dx, 2) for dy in range(-1, 2) for dx in range(-1, 2)]
    + [(2, dy, dx, 4) for dy in range(-1, 2) for dx in range(-1, 2)]
)
NTAP = len(TAPS)


def _tap_off(t, r0):
    _, dy, dx, dil = TAPS[t]
    return IMG_BASE + (PAD + r0 + dy * dil) * WP + dx * dil


def _branch_pairs(br):
    """Pair taps of one branch with matching dx-parity (even AP stride)."""
    idx = [t for t in range(NTAP) if TAPS[t][0] == br]
    even = [t for t in idx if (TAPS[t][2] * TAPS[t][3]) % 2 == 0]
    odd = [t for t in idx if (TAPS[t][2] * TAPS[t][3]) % 2 == 1]
    pairs = []
    for grp in (even, odd):
        for i in range(0, len(grp) - 1, 2):
            a, b = grp[i], grp[i + 1]
            if _tap_off(a, 0) > _tap_off(b, 0):
                a, b = b, a
            pairs.append((a, b))
        if len(grp) % 2:
            pairs.append((grp[-1], None))
    return pairs


PAIRS = {br: _branch_pairs(br) for br in range(3)}  # 13 + 5 + 5


def _pair_ap(base, delta):
    """[128, N] AP -> [128, 2, N] with dim-1 stride delta (even, >=0)."""
    assert delta >= 0 and delta % 2 == 0, delta
    u = base.unsqueeze(1).broadcast_to([base.shape[0], 2, base.shape[1]])
    raw = [list(p) for p in u.ap]
    raw[1][0] = delta
    u.ap = bass_rust.VecI64Pair([tuple(p) for p in raw])
    return u


_PROGRAM_CACHE = {}


def _build_program():
    nc = bacc.Bacc("TRN2", target_bir_lowering=False, debug=False,
                   num_devices=N_CORES)

    x4 = nc.dram_tensor("x4", [SAMPLES_PER_CORE, CIN, PIX], F16,
                        kind="ExternalInput").ap()
    wconv = nc.dram_tensor("wconv", [128, 128], F16,
                           kind="ExternalInput").ap()
    wout = nc.dram_tensor("wout", [128, 3 * COUT], BF16,
                          kind="ExternalInput").ap()
    aT_d = nc.dram_tensor("aT", [128, NTAP], F32, kind="ExternalInput").ap()
    bT_d = nc.dram_tensor("bT", [128, NTAP], F32, kind="ExternalInput").ap()
    ident_d = nc.dram_tensor("ident", [128, 128], F8,
                             kind="ExternalInput").ap()
    convb_d = nc.dram_tensor("convb", [128, 1], F32, kind="ExternalInput").ap()
    biasout_d = nc.dram_tensor("biasout", [128, 3], F32,
                               kind="ExternalInput").ap()
    y4 = nc.dram_tensor("y4", [SAMPLES_PER_CORE, COUT, PIX], F16,
                        kind="ExternalOutput").ap()

    with tile.TileContext(nc) as tc:
        with ExitStack() as ctx:
            consts = ctx.enter_context(tc.tile_pool(name="consts", bufs=1))
            xpool = ctx.enter_context(tc.tile_pool(name="xp", bufs=2))
            fpool = ctx.enter_context(tc.tile_pool(name="fp", bufs=1))
            opool = ctx.enter_context(tc.tile_pool(name="op", bufs=2))
            outpool = ctx.enter_context(tc.tile_pool(name="outp", bufs=2))
            smalls = ctx.enter_context(tc.tile_pool(name="sm", bufs=2))
            diagp = ctx.enter_context(tc.tile_pool(name="dg", bufs=2))
            ps_tap = ctx.enter_context(
                tc.tile_pool(name="pst", bufs=2, space="PSUM"))
            ps_tap2 = ctx.enter_context(
                tc.tile_pool(name="pst2", bufs=1, space="PSUM"))
            ps_out = ctx.enter_context(
                tc.tile_pool(name="pso", bufs=2, space="PSUM"))

            # padded-f tiles zeroed first (they gate the first conv evac);
            # const loads go on the ACT sequencer to keep Pool free.
            fpads = []
            for par in range(2):
                fp_t = fpool.tile([128, FPW], F8, tag=f"fpad{par}")
                nc.gpsimd.memset(fp_t[:], 0.0)
                fpads.append(fp_t)

            wconv_t = consts.tile([128, 128], F16, tag="wconv")
            nc.scalar.dma_start(wconv_t[:], wconv[:])
            wout_t = consts.tile([128, 3 * COUT], BF16, tag="wout")
            nc.scalar.dma_start(wout_t[:], wout[:])
            aT = consts.tile([128, NTAP], F32, tag="aT")
            nc.scalar.dma_start(aT[:], aT_d[:])
            bT = consts.tile([128, NTAP], F32, tag="bT")
            nc.scalar.dma_start(bT[:], bT_d[:])
            ident = consts.tile([128, 128], F8, tag="ident")
            nc.scalar.dma_start(ident[:], ident_d[:])
            convb = consts.tile([128, 1], F32, tag="convb")
            nc.scalar.dma_start(convb[:], convb_d[:])
            biasout = consts.tile([128, 3], F32, tag="biasout")
            nc.scalar.dma_start(biasout[:], biasout_d[:])

            def interior(fp_t, row0, nrows):
                v = fp_t[:, IMG_BASE:IMG_BASE + WP * WP].rearrange(
                    "p (r c) -> p r c", c=WP)
                return v[:, PAD + row0:PAD + row0 + nrows, PAD:PAD + W]

            # ---- phase 1: conv + g/ktile/diag for both blocks ----
            blk_state = {}
            for blk in range(SAMPLES_PER_CORE // 2):
                n0, n1 = 2 * blk, 2 * blk + 1
                fp_t = fpads[blk % 2]

                gsums = smalls.tile([128, NCHUNK], F32, tag=f"gsums{blk}")
                for q in range(PIX // SLAB):
                    xts = {}
                    for s, n in enumerate((n0, n1)):
                        for kc in range(2):
                            xt = xpool.tile([128, SLAB], F16, tag=f"x{s}{kc}")
                            nc.sync.dma_start(
                                xt[:],
                                x4[n, kc * 128:(kc + 1) * 128,
                                   q * SLAB:(q + 1) * SLAB])
                            xts[(s, kc)] = xt
                    for c in range(SLAB // CHUNK):
                        j = q * (SLAB // CHUNK) + c
                        ps = ps_tap.tile([128, CHUNK], F32, tag="o1")
                        for s in range(2):
                            for kc in range(2):
                                nc.tensor.matmul(
                                    ps[64 * s:64 * s + 64, :],
                                    wconv_t[:, kc * 64:(kc + 1) * 64],
                                    xts[(s, kc)][:, c * CHUNK:(c + 1) * CHUNK],
                                    start=(kc == 0), stop=(kc == 1))
                        dst = interior(fp_t, 8 * j, 8)
                        nc.scalar.activation(
                            dst, ps[:], AF.Identity, bias=convb[:, 0:1],
                            accum_out=gsums[:, j:j + 1])

                # relu over the interior (fp8, DVE 1x, in place)
                intr = interior(fp_t, 0, H)
                nc.vector.tensor_scalar_max(intr, intr, 0.0)

                gpre = smalls.tile([128, 1], F32, tag=f"gpre{blk}")
                nc.vector.tensor_reduce(gpre[:], gsums[:], op=ALU.add,
                                        axis=mybir.AxisListType.X)
                gt = smalls.tile([128, 1], F32, tag=f"g{blk}")
                nc.scalar.activation(gt[:], gpre[:], AF.Relu, scale=1.0 / PIX)
                ktile = smalls.tile([128, NTAP], F32, tag=f"ktile{blk}")
                nc.vector.scalar_tensor_tensor(ktile[:], aT[:], gt[:, 0:1],
                                               bT[:], op0=ALU.mult,
                                               op1=ALU.add)

                # diag pair tiles on Pool: [128, npairs*256] fp8
                npairs = sum(len(PAIRS[br]) for br in range(3))
                dgall = diagp.tile([128, npairs * 256], F8, tag="dg")
                i = 0
                pair_slots = {}
                for br in range(3):
                    for (ta, tb) in PAIRS[br]:
                        pair_slots[(br, ta, tb)] = i
                        eng_a = nc.vector if i % 3 != 2 else nc.gpsimd
                        eng_b = nc.vector if i % 3 == 0 else nc.gpsimd
                        eng_a.tensor_scalar_mul(
                            dgall[:, i * 256:i * 256 + 128], ident[:],
                            ktile[:, ta:ta + 1])
                        if tb is None:
                            nc.gpsimd.memset(
                                dgall[:, i * 256 + 128:(i + 1) * 256], 0.0)
                        else:
                            eng_b.tensor_scalar_mul(
                                dgall[:, i * 256 + 128:(i + 1) * 256],
                                ident[:], ktile[:, tb:tb + 1])
                        i += 1
                blk_state[blk] = (fp_t, dgall, pair_slots, n0, n1)

            # ---- phase 2: taps (PE fp8 DR) + output matmul per block ----
            for blk in range(SAMPLES_PER_CORE // 2):
                fp_t, dgall, pair_slots, n0, n1 = blk_state[blk]

                osb = {}
                for br in range(3):
                    ot = opool.tile([128, PIX], BF16, tag=f"o{br}")
                    osb[br] = ot

                for (r0, nr) in TSLABS:
                    sw = nr * WP
                    for br in range(3):
                        pool_br = ps_tap if br == 0 else ps_tap2
                        tps = pool_br.tile([128, CHUNK], F32,
                                           tag=f"o{br + 1}")
                        plist = PAIRS[br]
                        for ip, (ta, tb) in enumerate(plist):
                            slot = pair_slots[(br, ta, tb)]
                            off_a = _tap_off(ta, r0)
                            off_b = _tap_off(tb, r0) if tb is not None \
                                else off_a
                            lhsT = dgall[:, slot * 256:(slot + 1) * 256] \
                                .rearrange("p (two m) -> p two m", two=2)
                            rhs = _pair_ap(fp_t[:, off_a:off_a + sw],
                                           off_b - off_a)
                            nc.tensor.matmul(
                                tps[:, 0:sw], lhsT, rhs,
                                start=(ip == 0), stop=(ip == len(plist) - 1),
                                perf_mode=DRMODE)
                        # compaction evac: [128, nr, 64-of-72] -> o tile
                        pv = tps[:, 0:sw].rearrange("p (r c) -> p r c", c=WP)
                        src = pv[:, :, PAD:PAD + W]
                        dst = osb[br][:, r0 * W:(r0 + nr) * W].rearrange(
                            "p (r c) -> p r c", c=W)
                        if br == 0:
                            nc.scalar.activation(dst, src, AF.Copy)
                        else:
                            nc.vector.tensor_copy(dst, src)

                # output matmul: 3 pieces K=64 per (s, mt, chunk)
                OUTW = 1024
                stage = {}
                for c in range(NCHUNK):
                    half, cc = divmod(c, OUTW // CHUNK)
                    for mt in range(3):
                        for s in range(2):
                            ps = ps_out.tile([128, CHUNK], F32,
                                             tag=f"outps{s}")
                            for ipc in range(3):
                                lhsT = wout_t[64 * s:64 * s + 64,
                                              ipc * COUT + mt * 128:
                                              ipc * COUT + (mt + 1) * 128]
                                rhs = osb[ipc][64 * s:64 * s + 64,
                                               c * CHUNK:(c + 1) * CHUNK]
                                nc.tensor.matmul(ps[:], lhsT, rhs,
                                                 start=(ipc == 0),
                                                 stop=(ipc == 2))
                            if cc == 0:
                                st_new = outpool.tile(
                                    [128, OUTW], F16, tag=f"st{mt}_{s}")
                                stage[(half, mt, s)] = st_new
                            st = stage[(half, mt, s)]
                            sl = st[:, cc * CHUNK:(cc + 1) * CHUNK]
                            if s == 0:
                                nc.scalar.activation(
                                    sl, ps[:], AF.Identity,
                                    bias=biasout[:, mt:mt + 1])
                            else:
                                nc.vector.tensor_scalar_add(
                                    sl, ps[:], biasout[:, mt:mt + 1])
                            if cc == OUTW // CHUNK - 1:
                                n = (n0, n1)[s]
                                nc.sync.dma_start(
                                    y4[n, mt * 128:(mt + 1) * 128,
                                       half * OUTW:(half + 1) * OUTW],
                                    st[:])
    nc.compile()
    return nc


def _get_program():
    if "nc" not in _PROGRAM_CACHE:
        _PROGRAM_CACHE["nc"] = _build_program()
    return _PROGRAM_CACHE["nc"]


def kernel(x, conv_w, conv_b, ck_w, ck_b, ck2_w, ck2_b, ckd4_w, ckd4_b,
           kern_w, kern_b, kern2_w, kern2_b, kernd4_w, kernd4_b,
           fuse_w, fuse_b, fc_w, fc_b):
    import ml_dtypes
    x = np.asarray(x, dtype=np.float32)
    conv_w = np.asarray(conv_w, dtype=np.float32)
    conv_b = np.asarray(conv_b, dtype=np.float32)
    fuse_w = np.asarray(fuse_w, dtype=np.float32)
    fuse_b = np.asarray(fuse_b, dtype=np.float32)
    fc_w = np.asarray(fc_w, dtype=np.float32)
    fc_b = np.asarray(fc_b, dtype=np.float32)

    NB = x.shape[0]
    assert NB == N_CORES * SAMPLES_PER_CORE

    a1 = (float(ck_w) * np.asarray(kern_w)).astype(np.float32)
    b1 = (float(ck_w) * np.asarray(kern_b) + float(ck_b)).astype(np.float32)
    a2 = (float(ck2_w) * np.asarray(kern2_w)).astype(np.float32)
    b2 = (float(ck2_w) * np.asarray(kern2_b) + float(ck2_b)).astype(np.float32)
    a3 = (float(ckd4_w) * np.asarray(kernd4_w)).astype(np.float32)
    b3 = (float(ckd4_w) * np.asarray(kernd4_b) + float(ckd4_b)).astype(np.float32)
    a_all = np.concatenate([a1, a2, a3]).astype(np.float32)
    b_all = np.concatenate([b1, b2, b3]).astype(np.float32)
    aT = np.broadcast_to(a_all, (128, NTAP)).copy()
    bT = np.broadcast_to(b_all, (128, NTAP)).copy()

    Wi = [fc_w[:, 128 * i:128 * (i + 1)] @ fuse_w for i in range(3)]
    wout = np.zeros((128, 3 * COUT), dtype=np.float32)
    for i in range(3):
        wt = Wi[i].T.astype(np.float32)
        wout[0:64, i * COUT:(i + 1) * COUT] = wt
        wout[64:128, i * COUT:(i + 1) * COUT] = wt
    wout = wout.astype(ml_dtypes.bfloat16)
    bias_out = (fc_w @ np.tile(fuse_b, 3) + fc_b).astype(np.float32)
    biasout = bias_out.reshape(3, 128).T.copy()

    wconv = np.concatenate([conv_w[:, 0:128].T, conv_w[:, 128:256].T],
                           axis=1).astype(np.float16)
    convb = np.concatenate([conv_b, conv_b]).reshape(128, 1).astype(np.float32)
    ident = np.eye(128, dtype=np.float32).astype(ml_dtypes.float8_e4m3)

    x16 = x.reshape(NB, CIN, PIX).astype(np.float16)

    nc = _get_program()
    in_maps = []
    for core in range(N_CORES):
        xs = x16[core * SAMPLES_PER_CORE:(core + 1) * SAMPLES_PER_CORE]
        in_maps.append({
            "x4": np.ascontiguousarray(xs),
            "wconv": wconv, "wout": wout, "aT": aT, "bT": bT,
            "ident": ident, "convb": convb, "biasout": biasout,
        })
    res = run_bass_kernel_spmd(nc, in_maps, list(range(N_CORES)))
    out = np.empty((NB, COUT, H, W), dtype=np.float32)
    for core in range(N_CORES):
        out[core * SAMPLES_PER_CORE:(core + 1) * SAMPLES_PER_CORE] = (
            res.results[core]["y4"].astype(np.float32)
            .reshape(SAMPLES_PER_CORE, COUT, H, W))
    return out
